# revision 41
# baseline (speedup 1.0000x reference)
"""DispersionLoss Trainium2 kernel.

Computes mean over i<j pairs of exp(-||z_i - z_j||) for z [8192, 512] fp32,
distributed over 8 NeuronCores.

Strategy (identical SPMD program on all 8 cores; per-core behavior comes only
from input data):
  - Host rotates z by c*1024 rows for core c, so each core's 1024 rows sit at
    rotated positions 0..1023 and its circulant band of columns is the static
    range 0..5119. Each unordered pair {a,b} with forward separation
    s in [1, 4095] is computed exactly once somewhere in the fleet; the 4096
    pairs at separation exactly 4096 are added on the host in fp64.
  - Per core: 8 i-tiles x 9 j-units of [128, 512]. TensorE computes
    s = dot(z_i, z_j) - (sq_i + sq_j)/2 via 4 K=128 matmuls from a resident
    transposed band plus one K=1 augmented matmul (ones x -sq_j/2).
  - ScalarE, two passes in phase batches of 4 i-tiles (ACT program order is
    pinned so the sqrt/exp table sets load once per phase transition):
      pass1: d = Sqrt(-2*s + (sq_i + SQC + eps))  [PSUM -> fp16 d-buffer]
      pass2: e = Exp(-d), accum_out -> per-partition sums
  - GpSimd affine_select masks the band edges (j <= i and j - i >= 4096) by
    overwriting d with +200 so exp(-200) underflows to exactly 0.
  - Host: fp64 sum of the 8 cores' [128, 8] partial sums, undo the eps bias,
    add the separation-4096 pairs, divide by n(n-1)/2.
"""

import sys

sys.path.insert(0, "/opt/trn_rl_repo")

import numpy as np

N = 8192
D = 512
NCORES = 8
ROWS = N // NCORES          # 1024 rows per core
TI = ROWS // 128            # 8 i-tiles per core
NU = 9                      # j-units of 512 per i-tile
BANDC = (NU + 1) * 512      # 5120 resident band columns
EPS = 0.01                  # diagonal-safety shift added to d2 via the bias
SQC = 1024.0                # centering constant for the bf16 aug row
MASK_FILL = 200.0           # d fill for masked elements: exp(-200) == 0
TAU = 1.0

_CACHE = {}


def make_split_drain_tc():
    """TileContext subclass whose kernel-tail drain splits its sem waits into
    individual 1-wait NOPs (walrus rejects many waits on one instruction)."""
    from concourse import tile, mybir
    from concourse.vector_clock import ScopedClock

    class SplitDrainTileContext(tile.TileContext):
        def _drain_and_barrier(self, tick_clock, wait_clock):
            drain_inst = self.nc.sync.drain()
            wait_clock.add_sem_waits(
                drain_inst.ins, ScopedClock({None: tick_clock.global_clock})
            )
            si = drain_inst.ins.sync_info
            if si is not None and len(si.on_wait) > 1:
                waits = list(si.on_wait)
                drain_inst.ins.sync_info = mybir.SyncInfo(
                    on_wait=[], on_update=list(si.on_update)
                )
                for w in waits:
                    nop = self.nc.sync.nop(nofuse=True)
                    nop.ins.sync_info = mybir.SyncInfo(on_wait=[w], on_update=[])
            self.nc.all_engine_barrier()
            assert self.sems is not None
            popped = self.nc._tile_sem_poison_stack.pop()
            assert popped is self._sem_poison
            self.nc.clear_and_free_semaphores(list(self.sems.allocated().values()))
            self.nc.all_engine_barrier()

    return SplitDrainTileContext


def strip_pe_self_waits(nc):
    """Drop same-engine self-waits from compute instructions that carry more
    than one wait. Each engine executes and retires its own instruction stream
    in order (PE matmuls are pc-monotone; ACT/DVE are strict FIFO; GpSimd is
    per-Q7 FIFO with fixed partition ownership), so a wait on the instruction's
    own engine proc semaphore is redundant — and walrus can only encode one
    wait on most instruction structs."""
    import re

    from concourse import mybir

    eng_prefix = {
        mybir.EngineType.PE: "PE",
        mybir.EngineType.Activation: "Activation",
        mybir.EngineType.Pool: "Pool",
        mybir.EngineType.DVE: "DVE",
        mybir.EngineType.SP: "SP",
    }
    for f in nc.m.functions:
        for blk in f.blocks:
            new_insts = []
            for inst in blk.instructions:
                si = inst.sync_info
                if (
                    si is None
                    or len(si.on_wait) <= 1
                    or type(inst).__name__ == "InstDrain"
                ):
                    new_insts.append(inst)
                    continue
                keep = list(si.on_wait)
                name = eng_prefix.get(inst.engine)
                if name is not None:
                    pat = re.compile(rf"^{name}_\d+$")
                    keep = [w for w in keep if not pat.match(w.ant_name)]
                # Walrus encodes at most one wait on most instruction structs:
                # move extras onto same-engine NOPs issued just before (the
                # engine executes its queue in order, so waiting on the NOPs
                # first is equivalent).
                extras, keep = keep[1:], keep[:1]
                for w in extras:
                    nop = mybir.InstNoOp(
                        name=nc.get_next_instruction_name(),
                        ins=[],
                        outs=[],
                        engine=inst.engine,
                        sync_info=mybir.SyncInfo(on_wait=[w], on_update=[]),
                        bass_nofuse=True,
                    )
                    new_insts.append(nop)
                inst.sync_info = mybir.SyncInfo(
                    on_wait=keep, on_update=list(si.on_update)
                )
                new_insts.append(inst)
            blk.instructions = new_insts


def _build_nc():
    from concourse import bass, tile, mybir

    F32 = mybir.dt.float32
    BF16 = mybir.dt.bfloat16
    F16 = mybir.dt.float16
    AF = mybir.ActivationFunctionType
    SplitDrainTileContext = make_split_drain_tc()

    nc = bass.Bass()
    zT_d = nc.declare_dram_parameter("zT", [4, 128, BANDC], BF16, isOutput=False)
    nsqb_d = nc.declare_dram_parameter("nsqb", [128, BANDC], F16, isOutput=False)
    sqb_d = nc.declare_dram_parameter("sqb", [128, TI], F32, isOutput=False)
    out_d = nc.declare_dram_parameter("out", [128, TI], F32, isOutput=True)

    from concourse.tile import add_dep_helper

    with SplitDrainTileContext(nc) as tc:
        with (
            tc.tile_pool(name="zpool", bufs=1) as zpool,
            tc.tile_pool(name="small", bufs=1) as small,
            tc.tile_pool(name="dbuf", bufs=2) as dpool,
            tc.tile_pool(name="d2buf", bufs=8) as d2pool,
            tc.tile_pool(name="ebuf", bufs=1) as epool,
            tc.tile_pool(name="psum", bufs=3, space="PSUM") as pspool,
        ):
            # Each chunk split into tiles (per-tile DMA dependency granularity:
            # one DMA per tile keeps every matmul at <=1 DMA wait, and the
            # first tile is small enough that compute starts within ~5us).
            SPLITS = [0, 1024, 2048, 2560, BANDC]
            zTt = [
                [
                    zpool.tile(
                        [128, SPLITS[s + 1] - SPLITS[s]],
                        BF16,
                        name=f"zT{s}_{ch}",
                        tag=f"zT{s}_{ch}",
                    )
                    for s in range(4)
                ]
                for ch in range(4)
            ]
            nsqb = zpool.tile([128, BANDC], F16, name="nsqb", tag="nsqb")
            sqb = small.tile([128, TI], F32)
            acc = small.tile([128, TI], F32)
            wscr = small.tile([128, TI], F32)

            nc.sync.dma_start(nsqb[:], nsqb_d[:])
            for s in range(4):
                for ch in range(4):
                    nc.sync.dma_start(
                        zTt[ch][s][:], zT_d[ch, :, SPLITS[s] : SPLITS[s + 1]]
                    )
                if s == 0:
                    nc.sync.dma_start(sqb[:], sqb_d[:])

            def rhs_slice(ch, lo, width):
                for s in range(4):
                    if SPLITS[s] <= lo and lo + width <= SPLITS[s + 1]:
                        return zTt[ch][s][:, lo - SPLITS[s] : lo + width - SPLITS[s]]
                raise AssertionError(f"slice [{lo},{lo+width}) crosses tiles")

            # PE warmup: ~6us of dense dummy matmuls while the zT DMAs land,
            # so the HAM clock gate is already at 8/8 when real work starts.
            # Reads uninitialized SBUF (values irrelevant), writes a dedicated
            # PSUM slot that is never read.
            wsrc = zpool.tile([128, 512], BF16, name="wsrc", tag="wsrc")
            nc.vector.memset(wsrc[:], 0)
            wps = pspool.tile([128, 1024], F32, tag="ps")
            for _ in range(16):
                nc.tensor.matmul(
                    wps[:, 0:256], wsrc[:, 0:128], wsrc[:, 0:256], start=True,
                    stop=True, skip_group_check=True,
                )

            # Warmup: absorbs the sqb DMA wait into ACT and triggers the
            # sqrt table load immediately.
            act_chain = [nc.scalar.activation(wscr[:], sqb[:], AF.Sqrt)]

            def chained_act(*a, **kw):
                inst = nc.scalar.activation(*a, **kw)
                # Pin ACT program order (no-sync, same engine): keeps sqrt and
                # exp phases contiguous so walrus inserts exactly one
                # ACT_TABLE_LOAD per phase transition.
                add_dep_helper(inst.ins, act_chain[-1].ins, sync=False, reason="act order")
                act_chain.append(inst)
                return inst

            # Per i-tile the live j-window is exactly [128t+1, 128t+4224):
            # 4224 = 4095 separations + 128 diagonal cols + 1. PE computes
            # 512-aligned units except the narrowed first/last; DVE re-bases
            # everything onto the 4224-wide window (w = j - 128t), which also
            # makes the edge masks t-independent (keep w > p, keep w-p < 4096).
            W = 4224
            PHASES = [(0, 4), (4, 8)]
            dbufs = {}
            for t in range(TI):
                u0 = t // 4
                C = 128 * (t % 4)
                db = dpool.tile([128, W], F16, name=f"db{t}", tag=f"db{t % 4}")
                dbufs[t] = db
                d2 = d2pool.tile([128, W], F16, name=f"d2_{t}", tag="d2")

                # Five 512-aligned PSUM groups per i-tile (matmul outputs must
                # stay within one PSUM bank, so PE computes full units; the
                # window narrowing happens at the DVE read: group 0 keeps
                # ps[C:], the last group keeps only the first C+128 cols).
                for g in range(5):
                    g_lo = u0 * 512 + g * 1024
                    g_hi = min(g_lo + 1024, u0 * 512 + 4608)
                    ps = pspool.tile([128, 1024], F32, tag="ps")
                    for ch in range(4):
                        lhsT = zTt[ch][0][:, t * 128 : (t + 1) * 128]
                        for mm_lo in range(g_lo, g_hi, 512):
                            # narrow the dead window edges (bank-safe: the MM
                            # output still stays within its own PSUM bank)
                            n_lo, n_hi = mm_lo, mm_lo + 512
                            if g == 0 and mm_lo == g_lo:
                                n_lo = mm_lo + C
                            if g == 4 and mm_lo + 512 == g_hi:
                                n_hi = mm_lo + C + 128
                            if n_lo >= n_hi:
                                continue
                            nc.tensor.matmul(
                                ps[:, n_lo - g_lo : n_hi - g_lo],
                                lhsT,
                                rhs_slice(ch, n_lo, n_hi - n_lo),
                                start=(ch == 0),
                                stop=(ch == 3),
                                skip_group_check=True,
                            )
                    # live window within this group (w = j - 128t)
                    r_lo = C if g == 0 else 0
                    r_hi = (C + 128) if g == 4 else (g_hi - g_lo)
                    width = r_hi - r_lo
                    wbase = g * 1024 - C + r_lo
                    # DVE drains PSUM immediately (PE never waits on ACT) and
                    # fuses in the centered -sq_j/2 term.
                    nc.vector.tensor_add(
                        d2[:, wbase : wbase + width],
                        ps[:, r_lo:r_hi],
                        nsqb[:, g_lo + r_lo : g_lo + r_hi],
                    )
                    # d = sqrt(-2*(dot + c_j) + sq_i + SQC + EPS), per group
                    chained_act(
                        db[:, wbase : wbase + width],
                        d2[:, wbase : wbase + width],
                        AF.Sqrt,
                        bias=sqb[:, t : t + 1],
                        scale=-2.0,
                    )
                assert wbase + width == W

                # Band-edge masks on d (gpsimd): lower edge keeps w - p > 0
                nc.gpsimd.affine_select(
                    db[:, 0:256],
                    db[:, 0:256],
                    pattern=[[1, 256]],
                    channel_multiplier=-1,
                    base=0,
                    compare_op=mybir.AluOpType.is_gt,
                    fill=MASK_FILL,
                )
                # upper edge keeps w - p < 4096  <=>  p - (w - 4096) > 0
                nc.gpsimd.affine_select(
                    db[:, 4096:W],
                    db[:, 4096:W],
                    pattern=[[-1, 128]],
                    channel_multiplier=1,
                    base=0,
                    compare_op=mybir.AluOpType.is_gt,
                    fill=MASK_FILL,
                )

                for p0, p1 in PHASES:
                    if t != p1 - 1:
                        continue
                    # exp phase: e = exp(-d) with per-partition accumulation
                    for tt in range(p0, p1):
                        eb = epool.tile([128, W], BF16, tag="eb")
                        chained_act(
                            eb[:],
                            dbufs[tt][:],
                            AF.Exp,
                            scale=-1.0,
                            accum_out=acc[:, tt : tt + 1],
                        )

            nc.sync.dma_start(out_d[:], acc[:])

    strip_pe_self_waits(nc)
    return nc


def _enable_ldw_opt():
    """Flip walrus's --enable-ldw-opt to true: our matmul groups reuse the
    same stationary operand across consecutive MMs, and deduped LDWEIGHTS
    keeps the PE streaming instead of serializing LDW+MM."""
    if _CACHE.get("ldw_patched"):
        return
    from concourse import bass_utils

    orig = bass_utils.run_command

    def patched(cmd, *a, **kw):
        if isinstance(cmd, list):
            cmd = [
                "--enable-ldw-opt=true" if c == "--enable-ldw-opt=false" else c
                for c in cmd
            ]
        return orig(cmd, *a, **kw)

    bass_utils.run_command = patched
    _CACHE["ldw_patched"] = True


def _get_nc():
    if "nc" not in _CACHE:
        _CACHE["nc"] = _build_nc()
    return _CACHE["nc"]


def _make_in_maps(z: np.ndarray):
    import ml_dtypes

    zd = z.astype(np.float64)
    sq_full = (zd * zd).sum(axis=1)  # [N] fp64
    in_maps = []
    for c in range(NCORES):
        shift = c * ROWS
        rot = np.roll(z, -shift, axis=0)
        sq = np.roll(sq_full, -shift)
        band = rot[:BANDC]  # [BANDC, D]
        zT = (
            np.ascontiguousarray(band.T)
            .reshape(4, 128, BANDC)
            .astype(ml_dtypes.bfloat16)
        )
        # Centered so the fp16 d2 staging keeps ~0.1 absolute precision:
        # DVE adds c_j = (SQC - sq_j)/2; the SQC offset is restored via the
        # sqrt bias (sq_i + SQC).
        cj = (0.5 * (SQC - sq[:BANDC])).astype(ml_dtypes.float16 if hasattr(ml_dtypes, "float16") else np.float16)
        nsqb = np.ascontiguousarray(np.broadcast_to(cj, (128, BANDC)))
        sqb = (sq[:ROWS].reshape(TI, 128).T + SQC + EPS).astype(np.float32)
        in_maps.append({"zT": zT, "nsqb": nsqb, "sqb": sqb})
    return in_maps


def _run(z: np.ndarray, trace: bool = False):
    from concourse.bass_utils import run_bass_kernel_spmd

    nc = _get_nc()
    in_maps = _make_in_maps(z)
    res = run_bass_kernel_spmd(nc, in_maps, list(range(NCORES)), trace=trace)
    return res


def _postprocess(z: np.ndarray, results) -> np.float32:
    zd = z.astype(np.float64)
    total = 0.0
    for c in range(NCORES):
        total += float(results[c]["out"].astype(np.float64).sum())
    # Undo the EPS shift: d' = sqrt(d2+EPS) ~ d + EPS/(2d); dominant terms
    # have d ~ 32, so scale by exp(+EPS/64).
    total *= float(np.exp(EPS / 64.0))
    # Pairs at separation exactly 4096 (excluded on device), in fp64.
    diff = zd[: N // 2] - zd[N // 2 :]
    dsep = np.sqrt((diff * diff).sum(axis=1))
    total += float(np.exp(-dsep / TAU).sum())
    cnt = N * (N - 1) // 2
    return np.float32(total / cnt)


def kernel(z: np.ndarray) -> np.ndarray:
    z = np.ascontiguousarray(np.asarray(z, dtype=np.float32))
    assert z.shape == (N, D), z.shape
    res = _run(z, trace=False)
    return np.array(_postprocess(z, res.results), dtype=np.float32)


if __name__ == "__main__":
    rng = np.random.default_rng(0)
    z = rng.standard_normal((N, D)).astype(np.float32)
    print(kernel(z))


# revision 42
# speedup vs baseline: 1.0433x; 1.0433x over previous
"""DispersionLoss Trainium2 kernel.

Computes mean over i<j pairs of exp(-||z_i - z_j||) for z [8192, 512] fp32,
distributed over 8 NeuronCores.

Strategy (identical SPMD program on all 8 cores; per-core behavior comes only
from input data):
  - Host rotates z by c*1024 rows for core c, so each core's 1024 rows sit at
    rotated positions 0..1023 and its circulant band of columns is the static
    range 0..5119. Each unordered pair {a,b} with forward separation
    s in [1, 4095] is computed exactly once somewhere in the fleet; the 4096
    pairs at separation exactly 4096 are added on the host in fp64.
  - Per core: 8 i-tiles x 9 j-units of [128, 512]. TensorE computes
    s = dot(z_i, z_j) - (sq_i + sq_j)/2 via 4 K=128 matmuls from a resident
    transposed band plus one K=1 augmented matmul (ones x -sq_j/2).
  - ScalarE, two passes in phase batches of 4 i-tiles (ACT program order is
    pinned so the sqrt/exp table sets load once per phase transition):
      pass1: d = Sqrt(-2*s + (sq_i + SQC + eps))  [PSUM -> fp16 d-buffer]
      pass2: e = Exp(-d), accum_out -> per-partition sums
  - GpSimd affine_select masks the band edges (j <= i and j - i >= 4096) by
    overwriting d with +200 so exp(-200) underflows to exactly 0.
  - Host: fp64 sum of the 8 cores' [128, 8] partial sums, undo the eps bias,
    add the separation-4096 pairs, divide by n(n-1)/2.
"""

import sys

sys.path.insert(0, "/opt/trn_rl_repo")

import numpy as np

N = 8192
D = 512
NCORES = 8
ROWS = N // NCORES          # 1024 rows per core
TI = ROWS // 128            # 8 i-tiles per core
NU = 9                      # j-units of 512 per i-tile
BANDC = (NU + 1) * 512      # 5120 resident band columns
EPS = 0.01                  # diagonal-safety shift added to d2 via the bias
SQC = 1024.0                # centering constant for the bf16 aug row
MASK_FILL = 200.0           # d fill for masked elements: exp(-200) == 0
TAU = 1.0

_CACHE = {}


def make_split_drain_tc():
    """TileContext subclass whose kernel-tail drain splits its sem waits into
    individual 1-wait NOPs (walrus rejects many waits on one instruction)."""
    from concourse import tile, mybir
    from concourse.vector_clock import ScopedClock

    class SplitDrainTileContext(tile.TileContext):
        def _drain_and_barrier(self, tick_clock, wait_clock):
            drain_inst = self.nc.sync.drain()
            wait_clock.add_sem_waits(
                drain_inst.ins, ScopedClock({None: tick_clock.global_clock})
            )
            si = drain_inst.ins.sync_info
            if si is not None and len(si.on_wait) > 1:
                waits = list(si.on_wait)
                drain_inst.ins.sync_info = mybir.SyncInfo(
                    on_wait=[], on_update=list(si.on_update)
                )
                for w in waits:
                    nop = self.nc.sync.nop(nofuse=True)
                    nop.ins.sync_info = mybir.SyncInfo(on_wait=[w], on_update=[])
            self.nc.all_engine_barrier()
            assert self.sems is not None
            popped = self.nc._tile_sem_poison_stack.pop()
            assert popped is self._sem_poison
            self.nc.clear_and_free_semaphores(list(self.sems.allocated().values()))
            self.nc.all_engine_barrier()

    return SplitDrainTileContext


def strip_pe_self_waits(nc):
    """Drop same-engine self-waits from compute instructions that carry more
    than one wait. Each engine executes and retires its own instruction stream
    in order (PE matmuls are pc-monotone; ACT/DVE are strict FIFO; GpSimd is
    per-Q7 FIFO with fixed partition ownership), so a wait on the instruction's
    own engine proc semaphore is redundant — and walrus can only encode one
    wait on most instruction structs."""
    import re

    from concourse import mybir

    eng_prefix = {
        mybir.EngineType.PE: "PE",
        mybir.EngineType.Activation: "Activation",
        mybir.EngineType.Pool: "Pool",
        mybir.EngineType.DVE: "DVE",
        mybir.EngineType.SP: "SP",
    }
    for f in nc.m.functions:
        for blk in f.blocks:
            new_insts = []
            for inst in blk.instructions:
                si = inst.sync_info
                if (
                    si is None
                    or len(si.on_wait) <= 1
                    or type(inst).__name__ == "InstDrain"
                ):
                    new_insts.append(inst)
                    continue
                keep = list(si.on_wait)
                name = eng_prefix.get(inst.engine)
                if name is not None:
                    pat = re.compile(rf"^{name}_\d+$")
                    keep = [w for w in keep if not pat.match(w.ant_name)]
                # Walrus encodes at most one wait on most instruction structs:
                # move extras onto same-engine NOPs issued just before (the
                # engine executes its queue in order, so waiting on the NOPs
                # first is equivalent).
                extras, keep = keep[1:], keep[:1]
                for w in extras:
                    nop = mybir.InstNoOp(
                        name=nc.get_next_instruction_name(),
                        ins=[],
                        outs=[],
                        engine=inst.engine,
                        sync_info=mybir.SyncInfo(on_wait=[w], on_update=[]),
                        bass_nofuse=True,
                    )
                    new_insts.append(nop)
                inst.sync_info = mybir.SyncInfo(
                    on_wait=keep, on_update=list(si.on_update)
                )
                new_insts.append(inst)
            blk.instructions = new_insts


def _build_nc():
    from concourse import bass, tile, mybir

    F32 = mybir.dt.float32
    BF16 = mybir.dt.bfloat16
    F16 = mybir.dt.float16
    AF = mybir.ActivationFunctionType
    SplitDrainTileContext = make_split_drain_tc()

    nc = bass.Bass()
    zT_d = nc.declare_dram_parameter("zT", [4, 128, BANDC], BF16, isOutput=False)
    nsqb_d = nc.declare_dram_parameter("nsqb", [128, BANDC], F16, isOutput=False)
    sqb_d = nc.declare_dram_parameter("sqb", [128, TI], F32, isOutput=False)
    out_d = nc.declare_dram_parameter("out", [128, TI], F32, isOutput=True)

    from concourse.tile import add_dep_helper

    with SplitDrainTileContext(nc) as tc:
        with (
            tc.tile_pool(name="zpool", bufs=1) as zpool,
            tc.tile_pool(name="small", bufs=1) as small,
            tc.tile_pool(name="dbuf", bufs=2) as dpool,
            tc.tile_pool(name="d2buf", bufs=8) as d2pool,
            tc.tile_pool(name="ebuf", bufs=1) as epool,
            tc.tile_pool(name="psum", bufs=4, space="PSUM") as pspool,
        ):
            # Each chunk split into tiles (per-tile DMA dependency granularity:
            # one DMA per tile keeps every matmul at <=1 DMA wait, and the
            # first tile is small enough that compute starts within ~5us).
            SPLITS = [0, 1024, 2048, 2560, BANDC]
            zTt = [
                [
                    zpool.tile(
                        [128, SPLITS[s + 1] - SPLITS[s]],
                        BF16,
                        name=f"zT{s}_{ch}",
                        tag=f"zT{s}_{ch}",
                    )
                    for s in range(4)
                ]
                for ch in range(4)
            ]
            nsqb = zpool.tile([128, BANDC], F16, name="nsqb", tag="nsqb")
            sqb = small.tile([128, TI], F32)
            acc = small.tile([128, TI], F32)
            wscr = small.tile([128, TI], F32)

            nc.sync.dma_start(nsqb[:], nsqb_d[:])
            for s in range(4):
                for ch in range(4):
                    nc.sync.dma_start(
                        zTt[ch][s][:], zT_d[ch, :, SPLITS[s] : SPLITS[s + 1]]
                    )
                if s == 0:
                    nc.sync.dma_start(sqb[:], sqb_d[:])

            def rhs_slice(ch, lo, width):
                for s in range(4):
                    if SPLITS[s] <= lo and lo + width <= SPLITS[s + 1]:
                        return zTt[ch][s][:, lo - SPLITS[s] : lo + width - SPLITS[s]]
                raise AssertionError(f"slice [{lo},{lo+width}) crosses tiles")

            # PE warmup: ~6us of dense dummy matmuls while the zT DMAs land,
            # so the HAM clock gate is already at 8/8 when real work starts.
            # Reads uninitialized SBUF (values irrelevant), writes a dedicated
            # PSUM slot that is never read.
            wsrc = zpool.tile([128, 512], BF16, name="wsrc", tag="wsrc")
            nc.vector.memset(wsrc[:], 0)
            wps = pspool.tile([128, 1024], F32, tag="ps")
            for _ in range(16):
                nc.tensor.matmul(
                    wps[:, 0:256], wsrc[:, 0:128], wsrc[:, 0:256], start=True,
                    stop=True, skip_group_check=True,
                )

            # Warmup: absorbs the sqb DMA wait into ACT and triggers the
            # sqrt table load immediately.
            act_chain = [nc.scalar.activation(wscr[:], sqb[:], AF.Sqrt)]

            def chained_act(*a, **kw):
                inst = nc.scalar.activation(*a, **kw)
                # Pin ACT program order (no-sync, same engine): keeps sqrt and
                # exp phases contiguous so walrus inserts exactly one
                # ACT_TABLE_LOAD per phase transition.
                add_dep_helper(inst.ins, act_chain[-1].ins, sync=False, reason="act order")
                act_chain.append(inst)
                return inst

            # Per i-tile the live j-window is exactly [128t+1, 128t+4224):
            # 4224 = 4095 separations + 128 diagonal cols + 1. PE computes
            # 512-aligned units except the narrowed first/last; DVE re-bases
            # everything onto the 4224-wide window (w = j - 128t), which also
            # makes the edge masks t-independent (keep w > p, keep w-p < 4096).
            W = 4224
            PHASES = [(0, 4), (4, 8)]
            dbufs = {}
            for t in range(TI):
                u0 = t // 4
                C = 128 * (t % 4)
                db = dpool.tile([128, W], F16, name=f"db{t}", tag=f"db{t % 4}")
                dbufs[t] = db
                d2 = d2pool.tile([128, W], F16, name=f"d2_{t}", tag="d2")

                # Five 512-aligned PSUM groups per i-tile (matmul outputs must
                # stay within one PSUM bank, so PE computes full units; the
                # window narrowing happens at the DVE read: group 0 keeps
                # ps[C:], the last group keeps only the first C+128 cols).
                for g in range(5):
                    g_lo = u0 * 512 + g * 1024
                    g_hi = min(g_lo + 1024, u0 * 512 + 4608)
                    ps = pspool.tile([128, 1024], F32, tag="ps")
                    for ch in range(4):
                        lhsT = zTt[ch][0][:, t * 128 : (t + 1) * 128]
                        for mm_lo in range(g_lo, g_hi, 512):
                            # narrow the dead window edges (bank-safe: the MM
                            # output still stays within its own PSUM bank)
                            n_lo, n_hi = mm_lo, mm_lo + 512
                            if g == 0 and mm_lo == g_lo:
                                n_lo = mm_lo + C
                            if g == 4 and mm_lo + 512 == g_hi:
                                n_hi = mm_lo + C + 128
                            if n_lo >= n_hi:
                                continue
                            nc.tensor.matmul(
                                ps[:, n_lo - g_lo : n_hi - g_lo],
                                lhsT,
                                rhs_slice(ch, n_lo, n_hi - n_lo),
                                start=(ch == 0),
                                stop=(ch == 3),
                                skip_group_check=True,
                            )
                    # live window within this group (w = j - 128t)
                    r_lo = C if g == 0 else 0
                    r_hi = (C + 128) if g == 4 else (g_hi - g_lo)
                    width = r_hi - r_lo
                    wbase = g * 1024 - C + r_lo
                    # DVE drains PSUM immediately (PE never waits on ACT) and
                    # fuses in the centered -sq_j/2 term.
                    nc.vector.tensor_add(
                        d2[:, wbase : wbase + width],
                        ps[:, r_lo:r_hi],
                        nsqb[:, g_lo + r_lo : g_lo + r_hi],
                    )
                    # d = sqrt(-2*(dot + c_j) + sq_i + SQC + EPS), per group
                    chained_act(
                        db[:, wbase : wbase + width],
                        d2[:, wbase : wbase + width],
                        AF.Sqrt,
                        bias=sqb[:, t : t + 1],
                        scale=-2.0,
                    )
                assert wbase + width == W

                # Band-edge masks on d (gpsimd): lower edge keeps w - p > 0
                nc.gpsimd.affine_select(
                    db[:, 0:256],
                    db[:, 0:256],
                    pattern=[[1, 256]],
                    channel_multiplier=-1,
                    base=0,
                    compare_op=mybir.AluOpType.is_gt,
                    fill=MASK_FILL,
                )
                # upper edge keeps w - p < 4096  <=>  p - (w - 4096) > 0
                nc.gpsimd.affine_select(
                    db[:, 4096:W],
                    db[:, 4096:W],
                    pattern=[[-1, 128]],
                    channel_multiplier=1,
                    base=0,
                    compare_op=mybir.AluOpType.is_gt,
                    fill=MASK_FILL,
                )

                for p0, p1 in PHASES:
                    if t != p1 - 1:
                        continue
                    # exp phase: e = exp(-d) with per-partition accumulation
                    for tt in range(p0, p1):
                        eb = epool.tile([128, W], BF16, tag="eb")
                        chained_act(
                            eb[:],
                            dbufs[tt][:],
                            AF.Exp,
                            scale=-1.0,
                            accum_out=acc[:, tt : tt + 1],
                        )

            nc.sync.dma_start(out_d[:], acc[:])

    strip_pe_self_waits(nc)
    return nc


def _enable_ldw_opt():
    """Flip walrus's --enable-ldw-opt to true: our matmul groups reuse the
    same stationary operand across consecutive MMs, and deduped LDWEIGHTS
    keeps the PE streaming instead of serializing LDW+MM."""
    if _CACHE.get("ldw_patched"):
        return
    from concourse import bass_utils

    orig = bass_utils.run_command

    def patched(cmd, *a, **kw):
        if isinstance(cmd, list):
            cmd = [
                "--enable-ldw-opt=true" if c == "--enable-ldw-opt=false" else c
                for c in cmd
            ]
        return orig(cmd, *a, **kw)

    bass_utils.run_command = patched
    _CACHE["ldw_patched"] = True


def _get_nc():
    if "nc" not in _CACHE:
        _CACHE["nc"] = _build_nc()
    return _CACHE["nc"]


def _make_in_maps(z: np.ndarray):
    import ml_dtypes

    zd = z.astype(np.float64)
    sq_full = (zd * zd).sum(axis=1)  # [N] fp64
    in_maps = []
    for c in range(NCORES):
        shift = c * ROWS
        rot = np.roll(z, -shift, axis=0)
        sq = np.roll(sq_full, -shift)
        band = rot[:BANDC]  # [BANDC, D]
        zT = (
            np.ascontiguousarray(band.T)
            .reshape(4, 128, BANDC)
            .astype(ml_dtypes.bfloat16)
        )
        # Centered so the fp16 d2 staging keeps ~0.1 absolute precision:
        # DVE adds c_j = (SQC - sq_j)/2; the SQC offset is restored via the
        # sqrt bias (sq_i + SQC).
        cj = (0.5 * (SQC - sq[:BANDC])).astype(ml_dtypes.float16 if hasattr(ml_dtypes, "float16") else np.float16)
        nsqb = np.ascontiguousarray(np.broadcast_to(cj, (128, BANDC)))
        sqb = (sq[:ROWS].reshape(TI, 128).T + SQC + EPS).astype(np.float32)
        in_maps.append({"zT": zT, "nsqb": nsqb, "sqb": sqb})
    return in_maps


def _run(z: np.ndarray, trace: bool = False):
    from concourse.bass_utils import run_bass_kernel_spmd

    nc = _get_nc()
    in_maps = _make_in_maps(z)
    res = run_bass_kernel_spmd(nc, in_maps, list(range(NCORES)), trace=trace)
    return res


def _postprocess(z: np.ndarray, results) -> np.float32:
    zd = z.astype(np.float64)
    total = 0.0
    for c in range(NCORES):
        total += float(results[c]["out"].astype(np.float64).sum())
    # Undo the EPS shift: d' = sqrt(d2+EPS) ~ d + EPS/(2d); dominant terms
    # have d ~ 32, so scale by exp(+EPS/64).
    total *= float(np.exp(EPS / 64.0))
    # Pairs at separation exactly 4096 (excluded on device), in fp64.
    diff = zd[: N // 2] - zd[N // 2 :]
    dsep = np.sqrt((diff * diff).sum(axis=1))
    total += float(np.exp(-dsep / TAU).sum())
    cnt = N * (N - 1) // 2
    return np.float32(total / cnt)


def kernel(z: np.ndarray) -> np.ndarray:
    z = np.ascontiguousarray(np.asarray(z, dtype=np.float32))
    assert z.shape == (N, D), z.shape
    res = _run(z, trace=False)
    return np.array(_postprocess(z, res.results), dtype=np.float32)


if __name__ == "__main__":
    rng = np.random.default_rng(0)
    z = rng.standard_normal((N, D)).astype(np.float32)
    print(kernel(z))
